# revision 3
# baseline (speedup 1.0000x reference)
"""Bass/Trainium2 kernel for nn_LocalAttention (banded attention, window 16).

Self-contained: takes full inputs, shards over 8 NeuronCores as
(batch, head-octet, seq-half), runs a banded-attention Bass kernel per core,
gathers on host.

Math: reference zeroes out-of-band scores (not -inf) and softmaxes the FULL
row, so out-of-band entries contribute exp(0)=1.  With w = band_mask*exp(s):
  Z_i   = sum_window(w) + (S - 144)          (window = 144 keys, uniform)
  num_i = sum_window((w-1)*v) + sum_all(v)   (padded keys give w-1 = 0)
so only a 144-wide banded computation per 128-query block is needed.
"""
import os
import sys

for _p in ("/opt/trn_rl_repo",):
    if os.path.isdir(_p) and _p not in sys.path:
        sys.path.append(_p)

import numpy as np
import ml_dtypes

B, S, D = 2, 2048, 1024
H, HD = 16, 64
W = 16                    # band half-width 8
SC = 1024                 # seq chunk per core
HK = SC + W               # key halo chunk (1040)
HC = 512                  # head-dim columns per core (8 heads)
WIN = 144                 # key window per 128-query block
NBLK = SC // 128          # query blocks per head per core
NH = HC // HD             # heads per core

_CACHE = {}


def _build():
    import concourse.bacc as bacc
    import concourse.tile as tile
    from concourse import mybir

    f32 = mybir.dt.float32
    f32r = mybir.dt.float32r
    bf16 = mybir.dt.bfloat16

    nc = bacc.Bacc("TRN2", target_bir_lowering=False, debug=False, num_devices=8)

    xt = nc.dram_tensor("xt", [D, HK], f32r, kind="ExternalInput").ap()
    wq = nc.dram_tensor("wq", [D, HC], f32r, kind="ExternalInput").ap()
    wk = nc.dram_tensor("wk", [D, HC], f32r, kind="ExternalInput").ap()
    wv = nc.dram_tensor("wv", [D, HC], f32r, kind="ExternalInput").ap()
    wo = nc.dram_tensor("wo", [HC, D], f32r, kind="ExternalInput").ap()
    bq = nc.dram_tensor("bq", [HC], f32, kind="ExternalInput").ap()
    bk = nc.dram_tensor("bk", [HC], f32, kind="ExternalInput").ap()
    vsum = nc.dram_tensor("vsum", [1, HC], f32r, kind="ExternalInput").ap()
    ones = nc.dram_tensor("ones", [1, 128], f32r, kind="ExternalInput").ap()
    maskt = nc.dram_tensor("maskt", [128, WIN], f32, kind="ExternalInput").ap()
    id_bf = nc.dram_tensor("id_bf", [128, 128], bf16, kind="ExternalInput").ap()
    id_fr = nc.dram_tensor("id_fr", [128, 128], f32r, kind="ExternalInput").ap()
    out = nc.dram_tensor("out", [SC, D], f32, kind="ExternalOutput").ap()

    KD = D // 128     # 8 contraction tiles
    Exp = mybir.ActivationFunctionType.Exp

    with tile.TileContext(nc) as tc:
        with tc.tile_pool(name="w8", bufs=1) as w8, \
             tc.tile_pool(name="stat", bufs=1) as stat, \
             tc.tile_pool(name="acts", bufs=1) as acts, \
             tc.tile_pool(name="blk", bufs=3) as blk, \
             tc.tile_pool(name="sml", bufs=4) as sml, \
             tc.tile_pool(name="ob", bufs=3) as ob, \
             tc.tile_pool(name="pmm", bufs=3, space="PSUM") as pmm, \
             tc.tile_pool(name="ptr", bufs=3, space="PSUM") as ptr, \
             tc.tile_pool(name="pc", bufs=2, space="PSUM") as pc:

            # ---- static inputs -> SBUF ----
            xt_sb = stat.tile([128, KD, HK], f32r)
            xt_r = xt.rearrange("(o p) f -> p o f", p=128)
            for k in range(KD):
                nc.sync.dma_start(xt_sb[:, k], xt_r[:, k])
            wq_sb = stat.tile([128, KD, HC], f32r)
            wk_sb = stat.tile([128, KD, HC], f32r)
            wv_sb = stat.tile([128, KD, HC], f32r)
            for w_sb, w_dr in ((wq_sb, wq), (wk_sb, wk), (wv_sb, wv)):
                w_r = w_dr.rearrange("(o p) f -> p o f", p=128)
                for k in range(KD):
                    nc.sync.dma_start(w_sb[:, k], w_r[:, k])
            wo_sb = stat.tile([128, HC // 128, D], f32r)
            wo_r = wo.rearrange("(o p) f -> p o f", p=128)
            for k in range(HC // 128):
                nc.sync.dma_start(wo_sb[:, k], wo_r[:, k])
            bq_sb = stat.tile([128, HC // 128], f32)
            nc.sync.dma_start(bq_sb[:], bq.rearrange("(o p) -> p o", p=128))
            bk_sb = stat.tile([128, HC // 128], f32)
            nc.sync.dma_start(bk_sb[:], bk.rearrange("(o p) -> p o", p=128))
            vsum_sb = stat.tile([1, HC], f32r)
            nc.sync.dma_start(vsum_sb[:], vsum)
            ones_sb = stat.tile([1, 128], f32r)
            nc.sync.dma_start(ones_sb[:], ones)
            mask_sb = stat.tile([128, WIN], f32)
            nc.sync.dma_start(mask_sb[:], maskt)
            idbf_sb = stat.tile([128, 128], bf16)
            nc.sync.dma_start(idbf_sb[:], id_bf)
            idfr_sb = stat.tile([128, 128], f32r)
            nc.sync.dma_start(idfr_sb[:], id_fr)

            # standing broadcast of vsum across 128 partitions: ones^T @ vsum
            ps_vs = pmm.tile([128, 512], f32, tag="mm")
            nc.tensor.matmul(ps_vs[:, :HC], ones_sb[:], vsum_sb[:],
                             start=True, stop=True)
            vs_sb = stat.tile([128, HC], f32)
            nc.vector.tensor_copy(vs_sb[:], ps_vs[:, :HC])

            # ---- projections ----
            qt_sb = acts.tile([128, HC // 128, SC], bf16)   # Q^T (scaled by 1/8)
            kt_sb = acts.tile([128, HC // 128, HK], bf16)   # K^T over halo keys
            v_sb = acts.tile([128, (HK + 127) // 128, HC], bf16)  # V natural
            ctxt_sb = acts.tile([128, HC // 128, SC], f32r)  # ctx^T

            # Q^T[m, :] = (x @ Wq)^T, scaled; interior cols [8, 8+SC)
            for m in range(HC // 128):
                for nch in range(SC // 512):
                    ps = pmm.tile([128, 512], f32, tag="mm")
                    for k in range(KD):
                        nc.tensor.matmul(
                            ps[:], wq_sb[:, k, m * 128:(m + 1) * 128],
                            xt_sb[:, k, 8 + nch * 512: 8 + (nch + 1) * 512],
                            start=(k == 0), stop=(k == KD - 1))
                    nc.vector.tensor_scalar(
                        out=qt_sb[:, m, nch * 512:(nch + 1) * 512], in0=ps[:],
                        scalar1=0.125, scalar2=bq_sb[:, m:m + 1],
                        op0=mybir.AluOpType.mult, op1=mybir.AluOpType.add)

            # K^T over all HK halo keys
            k_chunks = [(0, 512), (512, 512), (1024, HK - 1024)]
            for m in range(HC // 128):
                for (c0, cw) in k_chunks:
                    ps = pmm.tile([128, 512], f32, tag="mm")
                    for k in range(KD):
                        nc.tensor.matmul(
                            ps[:, :cw], wk_sb[:, k, m * 128:(m + 1) * 128],
                            xt_sb[:, k, c0:c0 + cw],
                            start=(k == 0), stop=(k == KD - 1))
                    nc.vector.tensor_scalar(
                        out=kt_sb[:, m, c0:c0 + cw], in0=ps[:, :cw],
                        scalar1=bk_sb[:, m:m + 1], scalar2=None,
                        op0=mybir.AluOpType.add)

            # V natural [HK, HC]
            for mt in range((HK + 127) // 128):
                rows = min(128, HK - mt * 128)
                ps = pmm.tile([128, 512], f32, tag="mm")
                for k in range(KD):
                    nc.tensor.matmul(
                        ps[:rows, :HC],
                        xt_sb[:, k, mt * 128: mt * 128 + rows],
                        wv_sb[:, k, :], start=(k == 0), stop=(k == KD - 1))
                nc.vector.tensor_copy(v_sb[:rows, mt, :], ps[:rows, :HC])

            # ---- banded attention ----
            for h in range(NH):
                hp, hr = h // 2, (h % 2) * 64
                for t in range(NBLK):
                    q_ap = qt_sb[hr:hr + 64, hp, t * 128:(t + 1) * 128]
                    k_ap = kt_sb[hr:hr + 64, hp, t * 128: t * 128 + WIN]
                    ps_s = pmm.tile([128, 512], f32, tag="mm")
                    nc.tensor.matmul(ps_s[:, :WIN], q_ap, k_ap,
                                     start=True, stop=True)
                    w0 = blk.tile([128, WIN], f32, tag="w0")
                    nc.vector.tensor_tensor(w0[:], ps_s[:, :WIN], mask_sb[:],
                                            mybir.AluOpType.mult)
                    em = blk.tile([128, WIN], bf16, tag="em")
                    z = sml.tile([128, 1], f32, tag="z")
                    nc.scalar.activation(out=em[:], in_=w0[:], func=Exp,
                                         accum_out=z[:])
                    nc.vector.tensor_scalar_add(em[:], em[:], -1.0)
                    # transpose Em1 -> [WIN, 128] as two PE transposes
                    ps_t1 = ptr.tile([128, 128], bf16, tag="tr")
                    nc.tensor.transpose(ps_t1[:], em[:, 0:128], idbf_sb[:])
                    ps_t2 = ptr.tile([128, 128], bf16, tag="tr")
                    nc.tensor.transpose(ps_t2[:16, :], em[:, 128:WIN], idbf_sb[:])
                    et1 = blk.tile([128, 128], bf16, tag="et1")
                    nc.vector.tensor_copy(et1[:], ps_t1[:])
                    et2 = blk.tile([16, 128], bf16, tag="et2")
                    nc.vector.tensor_copy(et2[:], ps_t2[:16, :])
                    # ctx[q, hd] = sum_keys Em1^T v
                    ps_c = pc.tile([128, 64], f32, tag="c")
                    nc.tensor.matmul(ps_c[:], et1[:],
                                     v_sb[:, t, h * 64:(h + 1) * 64],
                                     start=True, stop=False)
                    nc.tensor.matmul(ps_c[:], et2[:],
                                     v_sb[0:16, t + 1, h * 64:(h + 1) * 64],
                                     start=False, stop=True)
                    # 1/Z, Z = z + (S - WIN)
                    rz = sml.tile([128, 1], f32, tag="rz")
                    nc.vector.tensor_scalar_add(rz[:], z[:], float(S - WIN))
                    nc.vector.reciprocal(rz[:], rz[:])
                    # ctx = (ps_c + vsum_bcast) * rz   -> f32r
                    cs = sml.tile([128, 64], f32, tag="cs")
                    nc.vector.tensor_tensor(cs[:], ps_c[:],
                                            vs_sb[:, h * 64:(h + 1) * 64],
                                            mybir.AluOpType.add)
                    cr = sml.tile([128, 64], f32r, tag="cr")
                    nc.vector.tensor_scalar_mul(cr[:], cs[:], rz[:])
                    # transpose ctx -> ctx^T [64, 128]
                    ps_ct = ptr.tile([128, 128], f32r, tag="tr")
                    nc.tensor.transpose(ps_ct[:64, :], cr[:], idfr_sb[:])
                    nc.vector.tensor_copy(
                        ctxt_sb[hr:hr + 64, hp, t * 128:(t + 1) * 128],
                        ps_ct[:64, :])

            # ---- out projection ----
            for st in range(SC // 128):
                for nch in range(D // 512):
                    ps = pmm.tile([128, 512], f32, tag="mm")
                    for kt in range(HC // 128):
                        nc.tensor.matmul(
                            ps[:], ctxt_sb[:, kt, st * 128:(st + 1) * 128],
                            wo_sb[:, kt, nch * 512:(nch + 1) * 512],
                            start=(kt == 0), stop=(kt == HC // 128 - 1))
                    o_sb = ob.tile([128, 512], f32)
                    nc.vector.tensor_copy(o_sb[:], ps[:])
                    nc.sync.dma_start(
                        out[st * 128:(st + 1) * 128, nch * 512:(nch + 1) * 512],
                        o_sb[:])

    nc.compile()
    return nc


def _get_nc():
    if "nc" not in _CACHE:
        _CACHE["nc"] = _build()
    return _CACHE["nc"]


LAST_EXEC_NS = None


def kernel(hidden_states, Wq, bq, Wk, bk, Wv, bv, Wo, bo):
    global LAST_EXEC_NS
    from concourse.bass_utils import run_bass_kernel_spmd

    hs = np.asarray(hidden_states, dtype=np.float32)
    Wq, Wk, Wv, Wo = (np.asarray(a, dtype=np.float32) for a in (Wq, Wk, Wv, Wo))
    bq, bk, bv, bo = (np.asarray(a, dtype=np.float32) for a in (bq, bk, bv, bo))

    xpad = np.zeros((B, S + W, D), np.float32)
    xpad[:, 8:8 + S] = hs
    xT = np.ascontiguousarray(xpad.transpose(0, 2, 1))  # [B, D, S+W]

    r = np.arange(WIN)[None, :]
    c = np.arange(128)[:, None]
    mask = ((r >= c) & (r <= c + W)).astype(np.float32)
    eye_bf = np.eye(128, dtype=ml_dtypes.bfloat16)

    in_maps = []
    for core in range(8):
        b, hg, sh = core // 4, (core // 2) % 2, core % 2
        cols = slice(hg * HC, (hg + 1) * HC)
        vs = xpad[b].sum(0, dtype=np.float64) @ Wv[:, cols].astype(np.float64)
        in_maps.append({
            "xt": np.ascontiguousarray(xT[b][:, sh * SC: sh * SC + HK]),
            "wq": np.ascontiguousarray(Wq[:, cols]),
            "wk": np.ascontiguousarray(Wk[:, cols]),
            "wv": np.ascontiguousarray(Wv[:, cols]),
            "wo": np.ascontiguousarray(Wo[cols, :]),
            "bq": np.ascontiguousarray(bq[cols] * np.float32(0.125)),
            "bk": np.ascontiguousarray(bk[cols]),
            "vsum": vs.astype(np.float32)[None, :],
            "ones": np.ones((1, 128), np.float32),
            "maskt": mask,
            "id_bf": eye_bf,
            "id_fr": np.eye(128, dtype=np.float32),
        })

    nc = _get_nc()
    trace_dir = os.environ.get("KERNEL_TRACE_DIR")
    kwargs = {}
    if trace_dir:
        kwargs = dict(trace=True, trace_cores=[0], tmpdir=trace_dir)
    res = run_bass_kernel_spmd(nc, in_maps, list(range(8)), **kwargs)
    LAST_EXEC_NS = res.exec_time_ns

    const = (bv.astype(np.float64) @ Wo.astype(np.float64)
             + bo.astype(np.float64)).astype(np.float32)
    outp = np.empty((B, S, D), np.float32)
    for b in range(B):
        for sh in range(2):
            acc = (res.results[4 * b + sh]["out"]
                   + res.results[4 * b + 2 + sh]["out"] + const)
            outp[b, sh * SC:(sh + 1) * SC] = acc
    return outp


# revision 5
# speedup vs baseline: 1.0133x; 1.0133x over previous
"""Bass/Trainium2 kernel for nn_LocalAttention (banded attention, window 16).

Self-contained: takes full inputs, shards over 8 NeuronCores as
(batch, head-octet, seq-half), runs a banded-attention Bass kernel per core,
gathers on host.

Math: the reference zeroes out-of-band scores (not -inf) and softmaxes the
FULL row, so out-of-band entries contribute exp(0)=1.  With
em1 = band_mask_applied(exp(s)) - 1 (exactly 0 off-band and on padded keys):
  Z_i   = sum_window(em1) + S
  num_i = sum_window(em1 * v) + sum_all(v)
so only a 144-wide banded computation per 128-query block is needed.
Scores are computed transposed ([keys, queries]) so em1 feeds the ctx matmul
as lhsT directly (no transposes); Z comes from an ones-column in V; 1/Z is
broadcast across partitions with a rank-1 matmul.  Biases bq/bk enter via an
augmented ones-row of x (zero on padded keys, so padding stays exact), and
bv/bo are folded on the host (softmax rows sum to 1).
"""
import os
import sys

for _p in ("/opt/trn_rl_repo",):
    if os.path.isdir(_p) and _p not in sys.path:
        sys.path.append(_p)

import numpy as np
import ml_dtypes

B, S, D = 2, 2048, 1024
H, HD = 16, 64
W = 16                    # band half-width 8
SC = 1024                 # seq chunk per core
HK = SC + W               # key halo chunk (1040)
HC = 512                  # head-dim columns per core (8 heads)
NBLK = SC // 128          # query blocks per head per core (8)
NH = HC // HD             # heads per core (8)
VST = 65                  # V stride per head in vaug (64 + ones col)

_CACHE = {}


def _build():
    import concourse.bacc as bacc
    import concourse.tile as tile
    from concourse import mybir

    f32 = mybir.dt.float32
    f32r = mybir.dt.float32r
    bf16 = mybir.dt.bfloat16

    nc = bacc.Bacc("TRN2", target_bir_lowering=False, debug=False, num_devices=8)

    xt = nc.dram_tensor("xt", [D, HK], f32r, kind="ExternalInput").ap()
    xa = nc.dram_tensor("xa", [1, HK], f32r, kind="ExternalInput").ap()
    wq = nc.dram_tensor("wq", [D, HC], f32r, kind="ExternalInput").ap()
    wk = nc.dram_tensor("wk", [D, HC], f32r, kind="ExternalInput").ap()
    wv = nc.dram_tensor("wv", [D, HC], f32r, kind="ExternalInput").ap()
    wo = nc.dram_tensor("wo", [HC, D], f32r, kind="ExternalInput").ap()
    bqr = nc.dram_tensor("bqr", [1, HC], f32r, kind="ExternalInput").ap()
    bkr = nc.dram_tensor("bkr", [1, HC], f32r, kind="ExternalInput").ap()
    vsum = nc.dram_tensor("vsum", [HC], f32, kind="ExternalInput").ap()
    ones = nc.dram_tensor("ones", [1, 128], f32r, kind="ExternalInput").ap()
    maskt = nc.dram_tensor("maskt", [128, 512], f32, kind="ExternalInput").ap()
    out = nc.dram_tensor("out", [SC, D], f32, kind="ExternalOutput").ap()

    KD = D // 128     # 8 contraction tiles
    Exp = mybir.ActivationFunctionType.Exp
    NVT = (HK + 127) // 128   # 9 V row tiles (last has 16 rows)

    with tile.TileContext(nc) as tc:
        with tc.tile_pool(name="stat", bufs=1) as stat, \
             tc.tile_pool(name="acts", bufs=1) as acts, \
             tc.tile_pool(name="blk", bufs=3) as blk, \
             tc.tile_pool(name="sml", bufs=4) as sml, \
             tc.tile_pool(name="ob", bufs=3) as ob, \
             tc.tile_pool(name="pmm", bufs=3, space="PSUM") as pmm, \
             tc.tile_pool(name="pst", bufs=2, space="PSUM") as pst, \
             tc.tile_pool(name="pcc", bufs=2, space="PSUM") as pcc, \
             tc.tile_pool(name="prz", bufs=1, space="PSUM") as prz:

            # ---- static inputs -> SBUF (spread over both HWDGE engines) ----
            xt_sb = stat.tile([128, KD, HK], f32r)
            xt_r = xt.rearrange("(o p) f -> p o f", p=128)
            for k in range(KD):
                nc.sync.dma_start(xt_sb[:, k], xt_r[:, k])
            xa_sb = stat.tile([1, HK], f32r)
            nc.sync.dma_start(xa_sb[:], xa)
            wq_sb = stat.tile([128, KD, HC], f32r)
            wk_sb = stat.tile([128, KD, HC], f32r)
            wv_sb = stat.tile([128, KD, HC], f32r)
            for w_sb, w_dr in ((wq_sb, wq), (wk_sb, wk), (wv_sb, wv)):
                w_r = w_dr.rearrange("(o p) f -> p o f", p=128)
                for k in range(KD):
                    nc.scalar.dma_start(w_sb[:, k], w_r[:, k])
            wo_sb = stat.tile([128, HC // 128, D], f32r)
            wo_r = wo.rearrange("(o p) f -> p o f", p=128)
            for k in range(HC // 128):
                nc.scalar.dma_start(wo_sb[:, k], wo_r[:, k])
            bqr_sb = stat.tile([1, HC], f32r)
            nc.sync.dma_start(bqr_sb[:], bqr)
            bkr_sb = stat.tile([1, HC], f32r)
            nc.sync.dma_start(bkr_sb[:], bkr)
            # vsum^T per head: [64, NH], head h at column h, partitions 0:64
            vsum_pc = stat.tile([64, NH], f32)
            nc.sync.dma_start(vsum_pc[:], vsum.rearrange("(h c) -> c h", c=64))
            # ones row at partition 0 (bias matmuls) and partition 64 (rz bcast)
            ones_sb = stat.tile([65, 128], f32r)
            nc.sync.dma_start(ones_sb[0:1, :], ones)
            nc.sync.dma_start(ones_sb[64:65, :], ones)
            mask_sb = stat.tile([128, 512], f32)
            nc.sync.dma_start(mask_sb[:], maskt)

            # ---- projections ----
            qt_sb = acts.tile([128, HC // 128, SC], bf16)   # Q^T (scaled 1/8)
            kt_sb = acts.tile([128, HC // 128, HK], bf16)   # K^T over halo keys
            vaug_sb = acts.tile([128, NVT, NH * VST], bf16)  # [V | 1] per head
            ctxt_sb = acts.tile([128, HC // 128, SC], f32r)  # ctx^T

            # ones column of vaug
            va_view = vaug_sb[:].rearrange("p a (h c) -> p a h c", c=VST)
            nc.gpsimd.memset(va_view[:, :, :, 64:65], 1.0)

            # Q^T = (x @ Wq + 1 x bq)^T * 0.125
            for m in range(HC // 128):
                for nch in range(SC // 512):
                    ps = pmm.tile([128, 512], f32, tag="mm")
                    for k in range(KD):
                        nc.tensor.matmul(
                            ps[:], wq_sb[:, k, m * 128:(m + 1) * 128],
                            xt_sb[:, k, 8 + nch * 512: 8 + (nch + 1) * 512],
                            start=(k == 0), stop=False)
                    nc.tensor.matmul(
                        ps[:], bqr_sb[0:1, m * 128:(m + 1) * 128],
                        xa_sb[0:1, 8 + nch * 512: 8 + (nch + 1) * 512],
                        start=False, stop=True)
                    nc.vector.tensor_scalar_mul(
                        qt_sb[:, m, nch * 512:(nch + 1) * 512], ps[:], 0.125)

            # K^T over all HK halo keys
            k_chunks = [(0, 512), (512, 512), (1024, HK - 1024)]
            for m in range(HC // 128):
                for (c0, cw) in k_chunks:
                    ps = pmm.tile([128, 512], f32, tag="mm")
                    for k in range(KD):
                        nc.tensor.matmul(
                            ps[:, :cw], wk_sb[:, k, m * 128:(m + 1) * 128],
                            xt_sb[:, k, c0:c0 + cw],
                            start=(k == 0), stop=False)
                    nc.tensor.matmul(
                        ps[:, :cw], bkr_sb[0:1, m * 128:(m + 1) * 128],
                        xa_sb[0:1, c0:c0 + cw], start=False, stop=True)
                    nc.vector.tensor_copy(kt_sb[:, m, c0:c0 + cw], ps[:, :cw])

            # V natural [HK, HC] -> vaug (stride-65 per head, col 64 = ones)
            for mt in range(NVT):
                rows = min(128, HK - mt * 128)
                ps = pmm.tile([128, 512], f32, tag="mm")
                for k in range(KD):
                    nc.tensor.matmul(
                        ps[:rows, :HC],
                        xt_sb[:, k, mt * 128: mt * 128 + rows],
                        wv_sb[:, k, :], start=(k == 0), stop=(k == KD - 1))
                nc.vector.tensor_copy(
                    va_view[:rows, mt, :, 0:64],
                    ps[:rows, :HC].rearrange("p (h c) -> p h c", c=64))

            # ---- banded attention ----
            # scores transposed [keys, queries]; batch 2 blocks per psum/ops,
            # 4 blocks per ctx psum/epilogue.
            for h in range(NH):
                hp, hr = h // 2, (h % 2) * 64
                for tt in range(NBLK // 4):      # super-block of 4 q-blocks
                    em_list = []
                    for half in range(2):        # 2 q-blocks per scores batch
                        psT = pst.tile([128, 512], f32, tag="st")
                        for j in range(2):
                            t = tt * 4 + half * 2 + j
                            nc.tensor.matmul(
                                psT[:, j * 256: j * 256 + 128],
                                kt_sb[hr:hr + 64, hp, t * 128: t * 128 + 128],
                                qt_sb[hr:hr + 64, hp, t * 128:(t + 1) * 128],
                                start=True, stop=True)
                            nc.tensor.matmul(
                                psT[0:16, j * 256 + 128: j * 256 + 256],
                                kt_sb[hr:hr + 64, hp,
                                      t * 128 + 128: t * 128 + 144],
                                qt_sb[hr:hr + 64, hp, t * 128:(t + 1) * 128],
                                start=True, stop=True)
                        w0 = blk.tile([128, 512], f32, tag="w0")
                        nc.vector.tensor_tensor(w0[:], psT[:], mask_sb[:],
                                                mybir.AluOpType.mult)
                        em = blk.tile([128, 512], bf16, tag="em")
                        nc.scalar.activation(out=em[:], in_=w0[:], func=Exp)
                        nc.vector.tensor_scalar_add(em[:], em[:], -1.0)
                        em_list.append(em)

                    # ctx^T+z for 4 blocks into one [65, 512] psum
                    ps_c = pcc.tile([65, 512], f32, tag="cc")
                    for q in range(4):
                        t = tt * 4 + q
                        em = em_list[q // 2]
                        off = (q % 2) * 256
                        nc.tensor.matmul(
                            ps_c[:, q * 128:(q + 1) * 128],
                            vaug_sb[:, t, h * VST:(h + 1) * VST],
                            em[:, off: off + 128], start=True, stop=False)
                        nc.tensor.matmul(
                            ps_c[:, q * 128:(q + 1) * 128],
                            vaug_sb[0:16, t + 1, h * VST:(h + 1) * VST],
                            em[0:16, off + 128: off + 256],
                            start=False, stop=True)
                    # rz = 1/(z + S) at partition 64, broadcast via rank-1 mm
                    zc = sml.tile([65, 512], f32, tag="zc")
                    rz = sml.tile([65, 512], f32r, tag="rz")
                    nc.vector.tensor_scalar_add(zc[64:65, :], ps_c[64:65, :],
                                                float(S))
                    with nc.allow_low_precision(reason="fp32r for rank-1 bcast"):
                        nc.vector.reciprocal(rz[64:65, :], zc[64:65, :])
                    ps_rz = prz.tile([64, 512], f32, tag="rzb")
                    nc.tensor.matmul(ps_rz[:], ones_sb[64:65, 0:64],
                                     rz[64:65, :], start=True, stop=True)
                    # ctx = (num + vsum) * rz
                    cs = sml.tile([64, 512], f32, tag="cs")
                    nc.vector.tensor_scalar_add(cs[:], ps_c[0:64, :],
                                                vsum_pc[:, h:h + 1])
                    if hr == 0:
                        nc.vector.tensor_tensor(
                            ctxt_sb[0:64, hp, tt * 512:(tt + 1) * 512],
                            cs[:], ps_rz[:], mybir.AluOpType.mult)
                    else:
                        cr = sml.tile([64, 512], f32r, tag="cr")
                        nc.vector.tensor_tensor(cr[:], cs[:], ps_rz[:],
                                                mybir.AluOpType.mult)
                        nc.sync.dma_start(
                            ctxt_sb[64:128, hp, tt * 512:(tt + 1) * 512],
                            cr[:])

            # ---- out projection ----
            for st in range(SC // 128):
                for nch in range(D // 512):
                    ps = pmm.tile([128, 512], f32, tag="mm")
                    for kt in range(HC // 128):
                        nc.tensor.matmul(
                            ps[:], ctxt_sb[:, kt, st * 128:(st + 1) * 128],
                            wo_sb[:, kt, nch * 512:(nch + 1) * 512],
                            start=(kt == 0), stop=(kt == HC // 128 - 1))
                    o_sb = ob.tile([128, 512], f32)
                    nc.vector.tensor_copy(o_sb[:], ps[:])
                    nc.sync.dma_start(
                        out[st * 128:(st + 1) * 128, nch * 512:(nch + 1) * 512],
                        o_sb[:])

    nc.compile()
    return nc


def _get_nc():
    if "nc" not in _CACHE:
        _CACHE["nc"] = _build()
    return _CACHE["nc"]


LAST_EXEC_NS = None


def _band_maskt():
    """[128, 512] f32: two copies of the transposed-window mask pair."""
    m = np.zeros((128, 512), np.float32)
    r = np.arange(128)[:, None]
    c = np.arange(128)[None, :]
    main = ((c <= r) & (r <= c + W)).astype(np.float32)   # keys 0..127
    r2 = np.arange(16)[:, None]
    tail = (c >= 112 + r2).astype(np.float32)             # keys 128..143
    for j in (0, 1):
        m[:, j * 256: j * 256 + 128] = main
        m[:16, j * 256 + 128: j * 256 + 256] = tail
    return m


def kernel(hidden_states, Wq, bq, Wk, bk, Wv, bv, Wo, bo):
    global LAST_EXEC_NS
    from concourse.bass_utils import run_bass_kernel_spmd

    hs = np.asarray(hidden_states, dtype=np.float32)
    Wq, Wk, Wv, Wo = (np.asarray(a, dtype=np.float32) for a in (Wq, Wk, Wv, Wo))
    bq, bk, bv, bo = (np.asarray(a, dtype=np.float32) for a in (bq, bk, bv, bo))

    xpad = np.zeros((B, S + W, D), np.float32)
    xpad[:, 8:8 + S] = hs
    xT = np.ascontiguousarray(xpad.transpose(0, 2, 1))  # [B, D, S+W]
    xav = np.zeros((B, S + W), np.float32)
    xav[:, 8:8 + S] = 1.0                               # ones row (0 on pads)

    maskt = _band_maskt()

    in_maps = []
    for core in range(8):
        b, hg, sh = core // 4, (core // 2) % 2, core % 2
        cols = slice(hg * HC, (hg + 1) * HC)
        vs = xpad[b].sum(0, dtype=np.float64) @ Wv[:, cols].astype(np.float64)
        in_maps.append({
            "xt": np.ascontiguousarray(xT[b][:, sh * SC: sh * SC + HK]),
            "xa": np.ascontiguousarray(xav[b][None, sh * SC: sh * SC + HK]),
            "wq": np.ascontiguousarray(Wq[:, cols]),
            "wk": np.ascontiguousarray(Wk[:, cols]),
            "wv": np.ascontiguousarray(Wv[:, cols]),
            "wo": np.ascontiguousarray(Wo[cols, :]),
            "bqr": np.ascontiguousarray(bq[None, cols]),
            "bkr": np.ascontiguousarray(bk[None, cols]),
            "vsum": vs.astype(np.float32),
            "ones": np.ones((1, 128), np.float32),
            "maskt": maskt,
        })

    nc = _get_nc()
    trace_dir = os.environ.get("KERNEL_TRACE_DIR")
    kwargs = {}
    if trace_dir:
        kwargs = dict(trace=True, trace_cores=[0], tmpdir=trace_dir)
    res = run_bass_kernel_spmd(nc, in_maps, list(range(8)), **kwargs)
    LAST_EXEC_NS = res.exec_time_ns

    const = (bv.astype(np.float64) @ Wo.astype(np.float64)
             + bo.astype(np.float64)).astype(np.float32)
    outp = np.empty((B, S, D), np.float32)
    for b in range(B):
        for sh in range(2):
            acc = (res.results[4 * b + sh]["out"]
                   + res.results[4 * b + 2 + sh]["out"] + const)
            outp[b, sh * SC:(sh + 1) * SC] = acc
    return outp


# revision 6
# speedup vs baseline: 1.1265x; 1.1118x over previous
"""Bass/Trainium2 kernel for nn_LocalAttention (banded attention, window 16).

Self-contained: takes full inputs, shards over 8 NeuronCores as
(batch, head-octet, seq-half), runs a banded-attention Bass kernel per core,
gathers on host.

Math: the reference zeroes out-of-band scores (not -inf) and softmaxes the
FULL row, so out-of-band entries contribute exp(0)=1.  With
em1 = band_mask_applied(exp(s)) - 1 (exactly 0 off-band and on padded keys):
  Z_i   = sum_window(em1) + S
  num_i = sum_window(em1 * v) + sum_all(v)
so only a 144-wide banded computation per 128-query block is needed.
Scores are computed transposed ([keys, queries]) so em1 feeds the ctx matmul
as lhsT directly (no transposes); Z comes from an ones-column in V; 1/Z is
broadcast across partitions with a rank-1 matmul.  Biases bq/bk enter via an
augmented ones-row of x (zero on padded keys, so padding stays exact), and
bv/bo are folded on the host (softmax rows sum to 1).
"""
import os
import sys

for _p in ("/opt/trn_rl_repo",):
    if os.path.isdir(_p) and _p not in sys.path:
        sys.path.append(_p)

import numpy as np
import ml_dtypes

B, S, D = 2, 2048, 1024
H, HD = 16, 64
W = 16                    # band half-width 8
SC = 1024                 # seq chunk per core
HK = SC + W               # key halo chunk (1040)
HC = 512                  # head-dim columns per core (8 heads)
NBLK = SC // 128          # query blocks per head per core (8)
NH = HC // HD             # heads per core (8)
VST = 64                  # V stride per head in vaug

_CACHE = {}


def _build():
    import concourse.bacc as bacc
    import concourse.tile as tile
    from concourse import mybir

    f32 = mybir.dt.float32
    f32r = mybir.dt.float32r
    bf16 = mybir.dt.bfloat16

    nc = bacc.Bacc("TRN2", target_bir_lowering=False, debug=False, num_devices=8)

    xt = nc.dram_tensor("xt", [D, HK], f32r, kind="ExternalInput").ap()
    xa = nc.dram_tensor("xa", [1, HK], f32r, kind="ExternalInput").ap()
    wq = nc.dram_tensor("wq", [D, HC], f32r, kind="ExternalInput").ap()
    wk = nc.dram_tensor("wk", [D, HC], f32r, kind="ExternalInput").ap()
    wv = nc.dram_tensor("wv", [D, HC], f32r, kind="ExternalInput").ap()
    wo = nc.dram_tensor("wo", [HC, D], f32r, kind="ExternalInput").ap()
    bqr = nc.dram_tensor("bqr", [1, HC], f32r, kind="ExternalInput").ap()
    bkr = nc.dram_tensor("bkr", [1, HC], f32r, kind="ExternalInput").ap()
    vsum = nc.dram_tensor("vsum", [HC], f32, kind="ExternalInput").ap()
    maskt = nc.dram_tensor("maskt", [128, 512], f32, kind="ExternalInput").ap()
    out = nc.dram_tensor("out", [SC, D], f32, kind="ExternalOutput").ap()

    KD = D // 128     # 8 contraction tiles
    Exp = mybir.ActivationFunctionType.Exp
    NVT = (HK + 127) // 128   # 9 V row tiles (last has 16 rows)

    with tile.TileContext(nc) as tc:
        with tc.tile_pool(name="stat", bufs=1) as stat, \
             tc.tile_pool(name="acts", bufs=1) as acts, \
             tc.tile_pool(name="blk", bufs=3) as blk, \
             tc.tile_pool(name="sml", bufs=4) as sml, \
             tc.tile_pool(name="ob", bufs=3) as ob, \
             tc.tile_pool(name="pmm", bufs=3, space="PSUM") as pmm, \
             tc.tile_pool(name="pst", bufs=2, space="PSUM") as pst, \
             tc.tile_pool(name="pcc", bufs=2, space="PSUM") as pcc, \
             tc.tile_pool(name="pzb", bufs=1, space="PSUM") as pzb:

            # ---- static inputs -> SBUF (spread over both HWDGE engines) ----
            xt_sb = stat.tile([128, KD, HK], f32r)
            xt_r = xt.rearrange("(o p) f -> p o f", p=128)
            for k in range(KD):
                nc.sync.dma_start(xt_sb[:, k], xt_r[:, k])
            xa_sb = stat.tile([1, HK], f32r)
            nc.sync.dma_start(xa_sb[:], xa)
            wq_sb = stat.tile([128, KD, HC], f32r)
            wk_sb = stat.tile([128, KD, HC], f32r)
            wv_sb = stat.tile([128, KD, HC], f32r)
            for w_sb, w_dr in ((wq_sb, wq), (wk_sb, wk), (wv_sb, wv)):
                w_r = w_dr.rearrange("(o p) f -> p o f", p=128)
                for k in range(KD):
                    nc.scalar.dma_start(w_sb[:, k], w_r[:, k])
            wo_sb = stat.tile([128, HC // 128, D], f32r)
            wo_r = wo.rearrange("(o p) f -> p o f", p=128)
            for k in range(HC // 128):
                nc.scalar.dma_start(wo_sb[:, k], wo_r[:, k])
            bqr_sb = stat.tile([1, HC], f32r)
            nc.sync.dma_start(bqr_sb[:], bqr)
            bkr_sb = stat.tile([1, HC], f32r)
            nc.sync.dma_start(bkr_sb[:], bkr)
            # vsum^T per head: [64, NH], head h at column h, partitions 0:64
            vsum_pc = stat.tile([64, NH], f32)
            nc.sync.dma_start(vsum_pc[:], vsum.rearrange("(h c) -> c h", c=64))
            mask_sb = stat.tile([128, 512], f32)
            nc.sync.dma_start(mask_sb[:], maskt)

            # ---- projections ----
            qt_sb = acts.tile([128, HC // 128, SC], bf16)   # Q^T (scaled 1/8)
            kt_sb = acts.tile([128, HC // 128, HK], bf16)   # K^T over halo keys
            vaug_sb = acts.tile([128, NVT, NH * VST], bf16)  # V per head
            ctxt_sb = acts.tile([128, HC // 128, SC], f32r)  # ctx^T
            onesm_sb = stat.tile([128, 64], bf16)            # zb matmul lhsT
            nc.gpsimd.memset(onesm_sb[:], 1.0)

            # Q^T = (x @ Wq + 1 x bq)^T * 0.125
            for m in range(HC // 128):
                for nch in range(SC // 512):
                    ps = pmm.tile([128, 512], f32, tag="mm")
                    for k in range(KD):
                        nc.tensor.matmul(
                            ps[:], wq_sb[:, k, m * 128:(m + 1) * 128],
                            xt_sb[:, k, 8 + nch * 512: 8 + (nch + 1) * 512],
                            start=(k == 0), stop=False)
                    nc.tensor.matmul(
                        ps[:], bqr_sb[0:1, m * 128:(m + 1) * 128],
                        xa_sb[0:1, 8 + nch * 512: 8 + (nch + 1) * 512],
                        start=False, stop=True)
                    nc.vector.tensor_scalar_mul(
                        qt_sb[:, m, nch * 512:(nch + 1) * 512], ps[:], 0.125)

            # K^T over all HK halo keys
            k_chunks = [(0, 512), (512, 512), (1024, HK - 1024)]
            for m in range(HC // 128):
                for (c0, cw) in k_chunks:
                    ps = pmm.tile([128, 512], f32, tag="mm")
                    for k in range(KD):
                        nc.tensor.matmul(
                            ps[:, :cw], wk_sb[:, k, m * 128:(m + 1) * 128],
                            xt_sb[:, k, c0:c0 + cw],
                            start=(k == 0), stop=False)
                    nc.tensor.matmul(
                        ps[:, :cw], bkr_sb[0:1, m * 128:(m + 1) * 128],
                        xa_sb[0:1, c0:c0 + cw], start=False, stop=True)
                    nc.vector.tensor_copy(kt_sb[:, m, c0:c0 + cw], ps[:, :cw])

            # V natural [HK, HC] -> vaug (stride-65 per head, col 64 = ones)
            for mt in range(NVT):
                rows = min(128, HK - mt * 128)
                ps = pmm.tile([128, 512], f32, tag="mm")
                for k in range(KD):
                    nc.tensor.matmul(
                        ps[:rows, :HC],
                        xt_sb[:, k, mt * 128: mt * 128 + rows],
                        wv_sb[:, k, :], start=(k == 0), stop=(k == KD - 1))
                nc.vector.tensor_copy(vaug_sb[:rows, mt, :], ps[:rows, :HC])

            # ---- banded attention ----
            # scores transposed [keys, queries]; batch 2 blocks per psum/ops,
            # 4 blocks per ctx psum/epilogue.
            for h in range(NH):
                hp, hr = h // 2, (h % 2) * 64
                for tt in range(NBLK // 4):      # super-block of 4 q-blocks
                    em_list = []
                    for half in range(2):        # 2 q-blocks per scores batch
                        psT = pst.tile([128, 512], f32, tag="st")
                        for j in range(2):
                            t = tt * 4 + half * 2 + j
                            nc.tensor.matmul(
                                psT[:, j * 256: j * 256 + 128],
                                kt_sb[hr:hr + 64, hp, t * 128: t * 128 + 128],
                                qt_sb[hr:hr + 64, hp, t * 128:(t + 1) * 128],
                                start=True, stop=True)
                            nc.tensor.matmul(
                                psT[0:16, j * 256 + 128: j * 256 + 256],
                                kt_sb[hr:hr + 64, hp,
                                      t * 128 + 128: t * 128 + 144],
                                qt_sb[hr:hr + 64, hp, t * 128:(t + 1) * 128],
                                start=True, stop=True)
                        w0 = blk.tile([128, 512], f32, tag="w0")
                        nc.vector.tensor_tensor(w0[:], psT[:], mask_sb[:],
                                                mybir.AluOpType.mult)
                        em = blk.tile([128, 512], bf16, tag="em")
                        nc.scalar.activation(out=em[:], in_=w0[:], func=Exp)
                        nc.vector.tensor_scalar_add(em[:], em[:], -1.0)
                        em_list.append(em)

                    # ctx^T num for 4 blocks into one [64, 512] psum, and
                    # Z broadcast [64, 512] via all-ones lhsT matmuls
                    ps_c = pcc.tile([64, 512], f32, tag="cc")
                    ps_z = pzb.tile([64, 512], f32, tag="zb")
                    for q in range(4):
                        t = tt * 4 + q
                        em = em_list[q // 2]
                        off = (q % 2) * 256
                        nc.tensor.matmul(
                            ps_c[:, q * 128:(q + 1) * 128],
                            vaug_sb[:, t, h * VST:(h + 1) * VST],
                            em[:, off: off + 128], start=True, stop=False)
                        nc.tensor.matmul(
                            ps_c[:, q * 128:(q + 1) * 128],
                            vaug_sb[0:16, t + 1, h * VST:(h + 1) * VST],
                            em[0:16, off + 128: off + 256],
                            start=False, stop=True)
                        nc.tensor.matmul(
                            ps_z[:, q * 128:(q + 1) * 128],
                            onesm_sb[:], em[:, off: off + 128],
                            start=True, stop=False)
                        nc.tensor.matmul(
                            ps_z[:, q * 128:(q + 1) * 128],
                            onesm_sb[0:16, :], em[0:16, off + 128: off + 256],
                            start=False, stop=True)
                    # ctx = (num + vsum) / (z + S), reciprocal via fast approx
                    zc = sml.tile([64, 512], f32, tag="zc")
                    nc.vector.tensor_scalar_add(zc[:], ps_z[:], float(S))
                    rz = sml.tile([64, 512], f32, tag="rz")
                    nc.vector.reciprocal_approx_fast(rz[:], zc[:])
                    cs = sml.tile([64, 512], f32, tag="cs")
                    nc.vector.tensor_scalar_add(cs[:], ps_c[:],
                                                vsum_pc[:, h:h + 1])
                    if hr == 0:
                        nc.vector.tensor_tensor(
                            ctxt_sb[0:64, hp, tt * 512:(tt + 1) * 512],
                            cs[:], rz[:], mybir.AluOpType.mult)
                    else:
                        cr = sml.tile([64, 512], f32r, tag="cr")
                        nc.vector.tensor_tensor(cr[:], cs[:], rz[:],
                                                mybir.AluOpType.mult)
                        nc.sync.dma_start(
                            ctxt_sb[64:128, hp, tt * 512:(tt + 1) * 512],
                            cr[:])

            # ---- out projection ----
            for st in range(SC // 128):
                for nch in range(D // 512):
                    ps = pmm.tile([128, 512], f32, tag="mm")
                    for kt in range(HC // 128):
                        nc.tensor.matmul(
                            ps[:], ctxt_sb[:, kt, st * 128:(st + 1) * 128],
                            wo_sb[:, kt, nch * 512:(nch + 1) * 512],
                            start=(kt == 0), stop=(kt == HC // 128 - 1))
                    o_sb = ob.tile([128, 512], f32)
                    nc.vector.tensor_copy(o_sb[:], ps[:])
                    nc.sync.dma_start(
                        out[st * 128:(st + 1) * 128, nch * 512:(nch + 1) * 512],
                        o_sb[:])

    nc.compile()
    return nc


def _get_nc():
    if "nc" not in _CACHE:
        _CACHE["nc"] = _build()
    return _CACHE["nc"]


LAST_EXEC_NS = None


def _band_maskt():
    """[128, 512] f32: two copies of the transposed-window mask pair."""
    m = np.zeros((128, 512), np.float32)
    r = np.arange(128)[:, None]
    c = np.arange(128)[None, :]
    main = ((c <= r) & (r <= c + W)).astype(np.float32)   # keys 0..127
    r2 = np.arange(16)[:, None]
    tail = (c >= 112 + r2).astype(np.float32)             # keys 128..143
    for j in (0, 1):
        m[:, j * 256: j * 256 + 128] = main
        m[:16, j * 256 + 128: j * 256 + 256] = tail
    return m


def kernel(hidden_states, Wq, bq, Wk, bk, Wv, bv, Wo, bo):
    global LAST_EXEC_NS
    from concourse.bass_utils import run_bass_kernel_spmd

    hs = np.asarray(hidden_states, dtype=np.float32)
    Wq, Wk, Wv, Wo = (np.asarray(a, dtype=np.float32) for a in (Wq, Wk, Wv, Wo))
    bq, bk, bv, bo = (np.asarray(a, dtype=np.float32) for a in (bq, bk, bv, bo))

    xpad = np.zeros((B, S + W, D), np.float32)
    xpad[:, 8:8 + S] = hs
    xT = np.ascontiguousarray(xpad.transpose(0, 2, 1))  # [B, D, S+W]
    xav = np.zeros((B, S + W), np.float32)
    xav[:, 8:8 + S] = 1.0                               # ones row (0 on pads)

    maskt = _band_maskt()

    in_maps = []
    for core in range(8):
        b, hg, sh = core // 4, (core // 2) % 2, core % 2
        cols = slice(hg * HC, (hg + 1) * HC)
        vs = xpad[b].sum(0, dtype=np.float64) @ Wv[:, cols].astype(np.float64)
        in_maps.append({
            "xt": np.ascontiguousarray(xT[b][:, sh * SC: sh * SC + HK]),
            "xa": np.ascontiguousarray(xav[b][None, sh * SC: sh * SC + HK]),
            "wq": np.ascontiguousarray(Wq[:, cols]),
            "wk": np.ascontiguousarray(Wk[:, cols]),
            "wv": np.ascontiguousarray(Wv[:, cols]),
            "wo": np.ascontiguousarray(Wo[cols, :]),
            "bqr": np.ascontiguousarray(bq[None, cols]),
            "bkr": np.ascontiguousarray(bk[None, cols]),
            "vsum": vs.astype(np.float32),
            "maskt": maskt,
        })

    nc = _get_nc()
    trace_dir = os.environ.get("KERNEL_TRACE_DIR")
    kwargs = {}
    if trace_dir:
        kwargs = dict(trace=True, trace_cores=[0], tmpdir=trace_dir)
    res = run_bass_kernel_spmd(nc, in_maps, list(range(8)), **kwargs)
    LAST_EXEC_NS = res.exec_time_ns

    const = (bv.astype(np.float64) @ Wo.astype(np.float64)
             + bo.astype(np.float64)).astype(np.float32)
    outp = np.empty((B, S, D), np.float32)
    for b in range(B):
        for sh in range(2):
            acc = (res.results[4 * b + sh]["out"]
                   + res.results[4 * b + 2 + sh]["out"] + const)
            outp[b, sh * SC:(sh + 1) * SC] = acc
    return outp
